# revision 47
# baseline (speedup 1.0000x reference)
"""BertSelfAttention on 8 TRN2 NeuronCores (Bass/Tile).

Sharding: core = (b, g) for b in 0..3 (batch), g in 0..1 (head group of 6
heads = 384 cols of the QKV projections). Pure SPMD, no collectives.

Per-core device kernel (all matmuls bf16, f32 PSUM):
  Projections: QT/KT [384, 2048] (d on partitions, head pairs per 128-row
  m-chunk) and V [2048, 384] (keys on partitions). Wq/bq are pre-scaled by
  log2e/8 on the host so scores come out in log2 units. V rows are scaled by
  e^mask on the cast and an e^mask denominator column is appended per head
  -> V_aug [128, 6*65].

  Attention, unit = (h, qc of 1024 q), 12 units, per kc (128 keys):
    scores psS [128k, 1024q] = kTh[:, kc].T @ qTh   (2 matmuls of 512)
    expS = 2^psS on ACT (Exp, scale=ln2). exp exists only on ACT in real
    codegen (pow is rejected on DVE/Pool; Pool runs at gpsimd software
    efficiency anyway), so ACT is the binding engine at ~201 us busy.
    ctx (flipped orientation), DEFERRED TWO UNITS (expS tiles buffered) so
    the PE stream never blocks ACT's exp supply:
      ctx_ps[bank][128q, 65] += expS[:, j*128:+128].T @ V_aug[kc][:, h*65:+65]
    (moving dim 65 instead of 1024 -> half the PE cost of the baseline).
  ACT (exp: 192 tiles x ~1.04 us = 199 us) is the binding engine; PE ~170 us.
  Projections ride as PE fillers with deadlines; DMAs are coarse host-packed
  tiles so SP dispatch (565 ns each) stays off the critical path.
  Drain: DVE copies ctx psum -> SBUF, DMA to DRAM [2048, 390]. Host divides
  by the denominator column and adds bv.
"""

import sys

for _p in ("/opt/trn_rl_repo",):
    if _p not in sys.path:
        sys.path.insert(0, _p)

import numpy as np
import ml_dtypes

import concourse.bass as bass  # noqa: F401
import concourse.mybir as mybir
from concourse import bacc, tile
from concourse.bass_utils import run_bass_kernel_spmd

AFT = mybir.ActivationFunctionType
ALU = mybir.AluOpType
BF16 = mybir.dt.bfloat16
F32 = mybir.dt.float32

B, S, H = 4, 2048, 768
NH, HD = 12, 64
N_CORES = 8
NH_LOC = 6
DL = NH_LOC * HD        # 384 local projection cols
KT = H // 128           # 6 k-tiles over hidden dim
M3 = DL // 128          # 3 m-chunks (head pairs)
KC = S // 128           # 16 key chunks
HDA = HD + 1            # head dim + denominator col
LN2 = float(np.log(2.0))
LOG2E = float(np.log2(np.e))
ACT_BIAS = 0.0          # uniform prob scale exp(ACT_BIAS); cancels in softmax

AUXW = M3 + M3 + KC + KC * NH_LOC   # bq | bk | em | em6 columns

_CACHED = None


def _build():
    nc = bacc.Bacc("TRN2", target_bir_lowering=False, debug=False,
                   num_devices=N_CORES)
    # host-packed inputs: xp[p, k*S+s] = x[s, k*128+p]; w*p[p, k*DL+c]
    xp = nc.dram_tensor("xp", [128, KT * S], BF16, kind="ExternalInput").ap()
    wqp = nc.dram_tensor("wqp", [128, KT * DL], BF16,
                         kind="ExternalInput").ap()
    wkp = nc.dram_tensor("wkp", [128, KT * DL], BF16,
                         kind="ExternalInput").ap()
    wvp = nc.dram_tensor("wvp", [128, KT * DL], BF16,
                         kind="ExternalInput").ap()
    aux = nc.dram_tensor("aux", [128, AUXW], F32, kind="ExternalInput").ap()
    outT = nc.dram_tensor("outT", [S, NH_LOC * HDA], F32,
                          kind="ExternalOutput").ap()

    with tile.TileContext(nc) as tc:
        with (
            tc.tile_pool(name="persist", bufs=1) as pp,
            tc.tile_pool(name="work", bufs=1) as wp,
            tc.tile_pool(name="psum", bufs=1, space="PSUM") as psp,
        ):
            # ---- persistent SBUF ----
            x_sb = pp.tile([128, KT * S], BF16, tag="x", name="x")
            wq_sb = pp.tile([128, KT * DL], BF16, tag="wq", name="wq")
            wk_sb = pp.tile([128, KT * DL], BF16, tag="wk", name="wk")
            wv_sb = pp.tile([128, KT * DL], BF16, tag="wv", name="wv")
            qT_t = [pp.tile([128, S], BF16, tag=f"qT{m}", name=f"qT{m}")
                    for m in range(M3)]
            kT_t = [pp.tile([128, S], BF16, tag=f"kT{m}", name=f"kT{m}")
                    for m in range(M3)]
            v_t = [pp.tile([128, NH_LOC * HDA], BF16, tag=f"v{s}",
                           name=f"v{s}") for s in range(KC)]
            aux_sb = pp.tile([128, AUXW], F32, tag="aux", name="aux")
            bq_sb = aux_sb[:, 0:M3]
            bk_sb = aux_sb[:, M3:2 * M3]
            em_sb = aux_sb[:, 2 * M3:2 * M3 + KC]
            em6_sb = aux_sb[:, 2 * M3 + KC:AUXW]
            abias = pp.tile([128, 1], F32, tag="abias", name="abias")

            # ---- coarse DMAs, front first ----
            def dma_x(c0, c1):
                nc.sync.dma_start(
                    x_sb[:].rearrange("p (k s) -> p k s", s=S)[:, :, c0:c1],
                    xp[:].rearrange("p (k s) -> p k s", s=S)[:, :, c0:c1])

            def dma_w(dst, src, c0, c1):
                nc.sync.dma_start(
                    dst[:].rearrange("p (k c) -> p k c", c=DL)[:, :, c0:c1],
                    src[:].rearrange("p (k c) -> p k c", c=DL)[:, :, c0:c1])

            # x cols are the long pole for the first projections (the DMA
            # device is serial): smallest chunks first, interleaved with the
            # small W/aux transfers the front projections also need.
            dma_x(0, 256)
            dma_w(wq_sb, wqp, 0, 128)        # Q m0
            dma_w(wk_sb, wkp, 0, 128)        # K m0
            dma_x(256, 512)
            nc.sync.dma_start(aux_sb[:], aux[:])
            dma_x(512, 1024)
            dma_w(wv_sb, wvp, 0, DL)
            dma_x(1024, 1536)
            dma_x(1536, 2048)
            dma_w(wq_sb, wqp, 128, DL)       # Q m1+m2
            dma_w(wk_sb, wkp, 128, DL)       # K m1+m2

            # PE p-state warm-up: junk matmuls while the first DMAs land, so
            # the front projections run at the full 2.4 GHz clock.
            wsc = pp.tile([128, 512], BF16, tag="wsc", name="wsc")
            nc.vector.memset(wsc[:], 0.0)
            nc.vector.memset(abias[:], ACT_BIAS)
            for wu in range(7):
                wps = psp.tile([128, 512], F32, tag="psP", bufs=2,
                               name=f"warm{wu}")
                nc.tensor.matmul(wps[:], wsc[:, 0:128], wsc[:],
                                 start=True, stop=True)

            def xs(k, c0, c1):
                return x_sb[:, k * S + c0:k * S + c1]

            def ws(w, k, c0, c1):
                return w[:, k * DL + c0:k * DL + c1]

            # ---- projections ----
            def proj_qk(dst, w_sb, b_sb, m, c0, c1):
                """Column chunk [c0:c1) (<=512 wide) of QT[m] / KT[m]."""
                ps = psp.tile([128, 512], F32, tag="psP", bufs=2,
                              name=f"pj{dst[m].name}_{c0}")
                for k in range(KT):
                    nc.tensor.matmul(
                        ps[:, 0:c1 - c0],
                        ws(w_sb, k, m * 128, (m + 1) * 128),
                        xs(k, c0, c1),
                        start=(k == 0), stop=(k == KT - 1))
                nc.vector.tensor_scalar(dst[m][:, c0:c1],
                                        ps[:, 0:c1 - c0], b_sb[:, m:m + 1],
                                        None, ALU.add)

            def proj_v(sc):
                """One 128-key chunk of V: scale rows by e^mask, append the
                e^mask denominator column per head."""
                ps = psp.tile([128, 512], F32, tag="psP", bufs=2,
                              name=f"pv{sc}")
                for k in range(KT):
                    nc.tensor.matmul(
                        ps[:, 0:DL],
                        xs(k, sc * 128, (sc + 1) * 128),
                        ws(wv_sb, k, 0, DL),
                        start=(k == 0), stop=(k == KT - 1))
                v3 = v_t[sc].rearrange("p (h e) -> p h e", e=HDA)
                nc.vector.tensor_scalar(
                    v3[:, :, 0:HD],
                    ps[:, 0:DL].rearrange("p (h e) -> p h e", e=HD),
                    em_sb[:, sc:sc + 1], None, ALU.mult)
                nc.gpsimd.tensor_copy(
                    v3[:, :, HD:HDA],
                    em6_sb[:, sc * NH_LOC:(sc + 1) * NH_LOC].rearrange(
                        "p (h e) -> p h e", e=1))

            # serial front: K m0 kc0-3 and the first q half-window; the
            # second q half (Qn1) is emitted inside the warm start, after the
            # kc0-3 qq0 half-scores, so PE doesn't stall on the x 512:1024
            # DMA before giving ACT its first exp work.
            proj_qk(kT_t, wk_sb, bk_sb, 0, 0, 256)
            proj_qk(qT_t, wq_sb, bq_sb, 0, 0, 512)

            # deferred PE fillers, scheduled at explicit (unit, step) slots so
            # the early units stay close to the ACT exp pace. Deadlines:
            # K m0 n_i by u0 step 4i; Q m0 n2/n3 by u1 s0; V sc by u2 step sc
            # (ctx deferral = 2); K/Q m1 n0-n1 by u4 s0, K m1 n2/n3 by u4
            # s8/s12, Q m1 n2/n3 by u5 s0; m2 likewise shifted +4 units.
            QK = "qk"
            slots = {}

            def put(u, s, item):
                slots.setdefault((u, s), []).append(item)

            put(0, 1, (QK, kT_t, wk_sb, bk_sb, 0, 512, 1024))
            put(0, 4, (QK, kT_t, wk_sb, bk_sb, 0, 1024, 1536))
            put(0, 8, (QK, kT_t, wk_sb, bk_sb, 0, 1536, 2048))
            put(0, 11, (QK, qT_t, wq_sb, bq_sb, 0, 1024, 1536))
            put(0, 13, (QK, qT_t, wq_sb, bq_sb, 0, 1536, 2048))
            for sc in range(9):                   # V0-8 in u1
                put(1, 1 + sc * 7 // 4, ("v", sc))
            for sc in range(9, KC):               # V9-15 JIT in u2
                put(2, 2 * (sc - 9), ("v", sc))
            for m, base_u in [(1, 3), (2, 6)]:
                put(base_u, 2, (QK, kT_t, wk_sb, bk_sb, m, 0, 512))
                put(base_u, 6, (QK, kT_t, wk_sb, bk_sb, m, 512, 1024))
                put(base_u, 10, (QK, qT_t, wq_sb, bq_sb, m, 0, 512))
                put(base_u, 14, (QK, qT_t, wq_sb, bq_sb, m, 512, 1024))
                put(base_u + 1, 2, (QK, kT_t, wk_sb, bk_sb, m, 1024, 1536))
                put(base_u + 1, 6, (QK, kT_t, wk_sb, bk_sb, m, 1536, 2048))
                put(base_u + 1, 10, (QK, qT_t, wq_sb, bq_sb, m, 1024, 1536))
                put(base_u + 1, 14, (QK, qT_t, wq_sb, bq_sb, m, 1536, 2048))

            def drain_slot(u, s):
                for item in slots.pop((u, s), ()):
                    if item[0] == "v":
                        proj_v(item[1])
                    else:
                        _, dst, w_sb, b_sb, m, c0, c1 = item
                        proj_qk(dst, w_sb, b_sb, m, c0, c1)

            # ---- attention: ctx deferred 2 units ----
            units = [(h, qc) for h in range(NH_LOC) for qc in range(2)]
            DEFER = 2
            exp_tiles = {}
            ctx_ps = {}

            def _half(u, kc, qq, psS, eo):
                h, qc = units[u]
                m, off = divmod(h, 2)
                off *= HD
                q0 = qc * 1024
                hs_ = slice(qq * 512, (qq + 1) * 512)
                nc.tensor.matmul(
                    psS[:, hs_],
                    kT_t[m][off:off + HD, kc * 128:(kc + 1) * 128],
                    qT_t[m][off:off + HD,
                            q0 + qq * 512:q0 + (qq + 1) * 512],
                    start=True, stop=True)
                nc.scalar.activation(eo[:, hs_], psS[:, hs_], AFT.Exp,
                                     bias=abias[:, 0:1], scale=LN2)

            def _alloc_skc(u, kc):
                psS = psp.tile([128, 1024], F32, tag="psS", bufs=2,
                               name=f"psS{u}_{kc}")
                eo = wp.tile([128, 1024], BF16, tag="eS", bufs=2 * KC + 8,
                             name=f"eS{u}_{kc}")
                exp_tiles[(u, kc)] = eo
                return psS, eo

            def emit_scores(u, kc):
                h, qc = units[u]
                m, off = divmod(h, 2)
                off *= HD
                q0 = qc * 1024
                psS, eo = _alloc_skc(u, kc)
                for qq in range(2):
                    nc.tensor.matmul(
                        psS[:, qq * 512:(qq + 1) * 512],
                        kT_t[m][off:off + HD, kc * 128:(kc + 1) * 128],
                        qT_t[m][off:off + HD,
                                q0 + qq * 512:q0 + (qq + 1) * 512],
                        start=True, stop=True)
                nc.scalar.activation(eo[:], psS[:], AFT.Exp,
                                     bias=abias[:, 0:1], scale=LN2)

            def emit_ctx(u, kc, tag=None):
                h, qc = units[u]
                if kc == 0:
                    for bank in range(2):
                        ctx_ps[(u, bank)] = psp.tile(
                            [128, 512], F32,
                            tag=tag or f"ctx{bank}",
                            bufs=2 if tag else 1,
                            name=f"ctx{u}_{bank}")
                eo = exp_tiles.pop((u, kc))
                for j in range(8):
                    bank, jj = divmod(j, 4)
                    nc.tensor.matmul(
                        ctx_ps[(u, bank)][:, jj * 128:jj * 128 + HDA],
                        eo[:, j * 128:(j + 1) * 128],
                        v_t[kc][:, h * HDA:(h + 1) * HDA],
                        start=(kc == 0 and jj == 0), stop=(kc == KC - 1))

            def emit_drain(u):
                h, qc = units[u]
                q0 = qc * 1024
                osb = wp.tile([128, 8 * HDA], F32, tag="osb", bufs=2,
                              name=f"osb{u}")
                o3 = osb.rearrange("p (j e) -> p j e", e=HDA)
                dst = outT[q0:q0 + 1024, h * HDA:(h + 1) * HDA].rearrange(
                    "(j p) e -> p j e", p=128)
                for bank in range(2):
                    src = ctx_ps.pop((u, bank)).rearrange(
                        "p (j e) -> p j e", e=128)
                    bs = slice(bank * 4, (bank + 1) * 4)
                    nc.vector.tensor_copy(o3[:, bs, :], src[:, :, 0:HDA])
                    nc.sync.dma_start(dst[:, bs, :], o3[:, bs, :])

            # Units 0..NU-1 run scores; ctx of unit u normally runs during
            # u+2 (ctx0/ctx1 psum banks). Tail compression: units NU-2 and
            # NU-1 each run TWO ctx streams -- the second in the projection
            # psum banks (free after unit NU-5) -- so the final unit's ctx
            # trails its own exps by ~3 kc instead of a full unit.
            NU = len(units)
            TRAIL = 3
            # unit-0 warm start: kc0/kc1 scores+exp in q-halves so ACT gets
            # its first work before the second q window's x columns land.
            # unit-0 warm start: kc0-3 scores+exp in q halves so ACT starts
            # as soon as the first q half's x columns land. kc2/3 qq0 borrow
            # the ctx psum banks (idle until unit 2).
            warm_eo = []
            for kc in range(4):
                eo = wp.tile([128, 1024], BF16, tag="eS", bufs=2 * KC + 8,
                             name=f"eS0_{kc}")
                exp_tiles[(0, kc)] = eo
                warm_eo.append(eo)

            def warm_half(kc, qq, ps_ap):
                nc.tensor.matmul(
                    ps_ap,
                    kT_t[0][0:HD, kc * 128:(kc + 1) * 128],
                    qT_t[0][0:HD, qq * 512:(qq + 1) * 512],
                    start=True, stop=True)
                nc.scalar.activation(warm_eo[kc][:, qq * 512:(qq + 1) * 512],
                                     ps_ap, AFT.Exp, bias=abias[:, 0:1],
                                     scale=LN2)

            wps01 = [psp.tile([128, 1024], F32, tag="psS", bufs=2,
                              name=f"wps{kc}") for kc in range(2)]
            warm_half(0, 0, wps01[0][:, 0:512])
            warm_half(1, 0, wps01[1][:, 0:512])
            proj_qk(kT_t, wk_sb, bk_sb, 0, 256, 512)       # K for kc2/3
            proj_qk(qT_t, wq_sb, bq_sb, 0, 512, 1024)      # Qn1
            wc2 = psp.tile([128, 512], F32, tag="ctx0", bufs=1, name="wc2a")
            warm_half(2, 0, wc2[:])
            wc3 = psp.tile([128, 512], F32, tag="ctx1", bufs=1, name="wc3a")
            warm_half(3, 0, wc3[:])
            warm_half(0, 1, wps01[0][:, 512:1024])
            warm_half(1, 1, wps01[1][:, 512:1024])
            wc2b = psp.tile([128, 512], F32, tag="ctx0", bufs=1, name="wc2b")
            warm_half(2, 1, wc2b[:])
            wc3b = psp.tile([128, 512], F32, tag="ctx1", bufs=1, name="wc3b")
            warm_half(3, 1, wc3b[:])
            for u in range(NU):
                for kc in range(KC):
                    if u == 0 and kc < 4:
                        pass        # emitted in the warm start above
                    else:
                        emit_scores(u, kc)
                    if DEFER <= u < NU - 2:
                        emit_ctx(u - DEFER, kc)
                    elif u == NU - 2:
                        emit_ctx(u - 2, kc)             # ctx(NU-4) in ctx0/1
                        emit_ctx(u - 1, kc, tag="psP")  # ctx(NU-3) in psP
                    elif u == NU - 1:
                        emit_ctx(u - 1, kc)             # ctx(NU-2) in ctx0/1
                        if kc >= TRAIL:
                            emit_ctx(u, kc - TRAIL, tag="psP")
                    drain_slot(u, kc)
                if DEFER <= u < NU - 2:
                    emit_drain(u - DEFER)
                elif u == NU - 2:
                    emit_drain(u - 2)
                    emit_drain(u - 1)
                elif u == NU - 1:
                    emit_drain(u - 1)
            for kc in range(KC - TRAIL, KC):
                emit_ctx(NU - 1, kc, tag="psP")
            emit_drain(NU - 1)
            assert not slots, f"unscheduled fillers: {list(slots)}"

    nc.compile()
    return nc


def _get_nc():
    global _CACHED
    if _CACHED is None:
        _CACHED = _build()
    return _CACHED


def kernel(hidden_states, attention_mask, Wq, bq, Wk, bk, Wv, bv):
    hidden_states = np.asarray(hidden_states, np.float32)
    attention_mask = np.asarray(attention_mask, np.float32)
    Wq, Wk, Wv = (np.asarray(w, np.float32) for w in (Wq, Wk, Wv))
    bq, bk, bv = (np.asarray(b, np.float32) for b in (bq, bk, bv))

    nc = _get_nc()
    sc = np.float32(LOG2E / np.sqrt(HD))
    in_maps = []
    for core in range(N_CORES):
        b, g = divmod(core, 2)
        cs = slice(g * DL, (g + 1) * DL)
        # xp[p, k*S+s] = hidden[b][s, k*128+p]
        xT = hidden_states[b].T.reshape(KT, 128, S)           # [k, p, s]
        xp = np.ascontiguousarray(xT.transpose(1, 0, 2).reshape(128, KT * S))

        def wpack(Wmat, scale=None):
            wT = Wmat[cs, :].T                                 # [H, DL]
            if scale is not None:
                wT = wT * scale
            w3 = wT.reshape(KT, 128, DL).transpose(1, 0, 2)    # [p, k, c]
            return np.ascontiguousarray(w3.reshape(128, KT * DL)).astype(
                ml_dtypes.bfloat16)

        em = np.exp(attention_mask[b, 0, 0, :]).astype(np.float32)
        emc = np.ascontiguousarray(em.reshape(KC, 128).T)      # [128, KC]
        em6 = np.repeat(emc[:, :, None], NH_LOC, axis=2).reshape(
            128, KC * NH_LOC)
        auxm = np.concatenate([
            (bq[cs] * sc).reshape(M3, 128).T,
            bk[cs].reshape(M3, 128).T,
            emc, em6], axis=1).astype(np.float32)
        in_maps.append({
            "xp": xp.astype(ml_dtypes.bfloat16),
            "wqp": wpack(Wq, sc),
            "wkp": wpack(Wk),
            "wvp": wpack(Wv),
            "aux": np.ascontiguousarray(auxm),
        })

    res = run_bass_kernel_spmd(nc, in_maps, core_ids=list(range(N_CORES)))

    out = np.empty((B, S, H), np.float32)
    for core in range(N_CORES):
        b, g = divmod(core, 2)
        oT = res.results[core]["outT"]              # [2048, 390]
        o3 = oT.reshape(S, NH_LOC, HDA)
        ctx = o3[:, :, 0:HD] / o3[:, :, HD:HDA]     # [S, 6, 64]
        cols = slice(g * DL, (g + 1) * DL)
        out[b, :, cols] = ctx.reshape(S, DL) + bv[cols][None, :]
    return out


# revision 50
# speedup vs baseline: 1.0001x; 1.0001x over previous
"""BertSelfAttention on 8 TRN2 NeuronCores (Bass/Tile).

Sharding: core = (b, g) for b in 0..3 (batch), g in 0..1 (head group of 6
heads = 384 cols of the QKV projections). Pure SPMD, no collectives.

Per-core device kernel (all matmuls bf16, f32 PSUM):
  Projections: QT/KT [384, 2048] (d on partitions, head pairs per 128-row
  m-chunk) and V [2048, 384] (keys on partitions). Wq/bq are pre-scaled by
  log2e/8 on the host so scores come out in log2 units. V rows are scaled by
  e^mask on the cast and an e^mask denominator column is appended per head
  -> V_aug [128, 6*65].

  Attention, unit = (h, qc of 1024 q), 12 units, per kc (128 keys):
    scores psS [128k, 1024q] = kTh[:, kc].T @ qTh   (2 matmuls of 512)
    expS = 2^psS on ACT (Exp, scale=ln2). exp exists only on ACT in real
    codegen (pow is rejected on DVE/Pool; Pool runs at gpsimd software
    efficiency anyway), so ACT is the binding engine at ~201 us busy.
    ctx (flipped orientation), DEFERRED TWO UNITS (expS tiles buffered) so
    the PE stream never blocks ACT's exp supply:
      ctx_ps[bank][128q, 65] += expS[:, j*128:+128].T @ V_aug[kc][:, h*65:+65]
    (moving dim 65 instead of 1024 -> half the PE cost of the baseline).
  ACT (exp: 192 tiles x ~1.04 us = 199 us) is the binding engine; PE ~170 us.
  Projections ride as PE fillers with deadlines; DMAs are coarse host-packed
  tiles so SP dispatch (565 ns each) stays off the critical path.
  Drain: DVE copies ctx psum -> SBUF, DMA to DRAM [2048, 390]. Host divides
  by the denominator column and adds bv.
"""

import sys

for _p in ("/opt/trn_rl_repo",):
    if _p not in sys.path:
        sys.path.insert(0, _p)

import numpy as np
import ml_dtypes

import concourse.bass as bass  # noqa: F401
import concourse.mybir as mybir
from concourse import bacc, tile
from concourse.bass_utils import run_bass_kernel_spmd

AFT = mybir.ActivationFunctionType
ALU = mybir.AluOpType
BF16 = mybir.dt.bfloat16
F32 = mybir.dt.float32

B, S, H = 4, 2048, 768
NH, HD = 12, 64
N_CORES = 8
NH_LOC = 6
DL = NH_LOC * HD        # 384 local projection cols
KT = H // 128           # 6 k-tiles over hidden dim
M3 = DL // 128          # 3 m-chunks (head pairs)
KC = S // 128           # 16 key chunks
HDA = HD + 1            # head dim + denominator col
LN2 = float(np.log(2.0))
LOG2E = float(np.log2(np.e))
ACT_BIAS = 0.0          # uniform prob scale exp(ACT_BIAS); cancels in softmax

AUXW = M3 + M3 + KC + KC * NH_LOC   # bq | bk | em | em6 columns

_CACHED = None


def _build():
    nc = bacc.Bacc("TRN2", target_bir_lowering=False, debug=False,
                   num_devices=N_CORES)
    # host-packed inputs: xp[p, k*S+s] = x[s, k*128+p]; w*p[p, k*DL+c]
    xp = nc.dram_tensor("xp", [128, KT * S], BF16, kind="ExternalInput").ap()
    wqp = nc.dram_tensor("wqp", [128, KT * DL], BF16,
                         kind="ExternalInput").ap()
    wkp = nc.dram_tensor("wkp", [128, KT * DL], BF16,
                         kind="ExternalInput").ap()
    wvp = nc.dram_tensor("wvp", [128, KT * DL], BF16,
                         kind="ExternalInput").ap()
    aux = nc.dram_tensor("aux", [128, AUXW], F32, kind="ExternalInput").ap()
    outT = nc.dram_tensor("outT", [S, NH_LOC * HDA], F32,
                          kind="ExternalOutput").ap()

    with tile.TileContext(nc) as tc:
        with (
            tc.tile_pool(name="persist", bufs=1) as pp,
            tc.tile_pool(name="work", bufs=1) as wp,
            tc.tile_pool(name="psum", bufs=1, space="PSUM") as psp,
        ):
            # ---- persistent SBUF ----
            x_sb = pp.tile([128, KT * S], BF16, tag="x", name="x")
            wq_sb = pp.tile([128, KT * DL], BF16, tag="wq", name="wq")
            wk_sb = pp.tile([128, KT * DL], BF16, tag="wk", name="wk")
            wv_sb = pp.tile([128, KT * DL], BF16, tag="wv", name="wv")
            qT_t = [pp.tile([128, S], BF16, tag=f"qT{m}", name=f"qT{m}")
                    for m in range(M3)]
            kT_t = [pp.tile([128, S], BF16, tag=f"kT{m}", name=f"kT{m}")
                    for m in range(M3)]
            v_t = [pp.tile([128, NH_LOC * HDA], BF16, tag=f"v{s}",
                           name=f"v{s}") for s in range(KC)]
            aux_sb = pp.tile([128, AUXW], F32, tag="aux", name="aux")
            bq_sb = aux_sb[:, 0:M3]
            bk_sb = aux_sb[:, M3:2 * M3]
            em_sb = aux_sb[:, 2 * M3:2 * M3 + KC]
            em6_sb = aux_sb[:, 2 * M3 + KC:AUXW]
            abias = pp.tile([128, 1], F32, tag="abias", name="abias")

            # ---- coarse DMAs, front first ----
            def dma_x(c0, c1):
                nc.sync.dma_start(
                    x_sb[:].rearrange("p (k s) -> p k s", s=S)[:, :, c0:c1],
                    xp[:].rearrange("p (k s) -> p k s", s=S)[:, :, c0:c1])

            def dma_w(dst, src, c0, c1):
                nc.sync.dma_start(
                    dst[:].rearrange("p (k c) -> p k c", c=DL)[:, :, c0:c1],
                    src[:].rearrange("p (k c) -> p k c", c=DL)[:, :, c0:c1])

            # x cols are the long pole for the first projections (the DMA
            # device is serial): smallest chunks first, interleaved with the
            # small W/aux transfers the front projections also need.
            dma_x(0, 256)
            dma_w(wk_sb, wkp, 0, 128)        # K m0 (first projection)
            dma_x(256, 512)
            dma_w(wq_sb, wqp, 0, 128)        # Q m0
            nc.sync.dma_start(aux_sb[:], aux[:])
            dma_x(512, 1024)
            dma_w(wv_sb, wvp, 0, DL)
            dma_x(1024, 1536)
            dma_x(1536, 2048)
            dma_w(wq_sb, wqp, 128, DL)       # Q m1+m2
            dma_w(wk_sb, wkp, 128, DL)       # K m1+m2

            # PE p-state warm-up: junk matmuls while the first DMAs land, so
            # the front projections run at the full 2.4 GHz clock.
            wsc = pp.tile([128, 512], BF16, tag="wsc", name="wsc")
            nc.vector.memset(wsc[:], 0.0)
            nc.vector.memset(abias[:], ACT_BIAS)
            for wu in range(7):
                wps = psp.tile([128, 512], F32, tag="psP", bufs=2,
                               name=f"warm{wu}")
                nc.tensor.matmul(wps[:], wsc[:, 0:128], wsc[:],
                                 start=True, stop=True)

            def xs(k, c0, c1):
                return x_sb[:, k * S + c0:k * S + c1]

            def ws(w, k, c0, c1):
                return w[:, k * DL + c0:k * DL + c1]

            # ---- projections ----
            def proj_qk(dst, w_sb, b_sb, m, c0, c1):
                """Column chunk [c0:c1) (<=512 wide) of QT[m] / KT[m]."""
                ps = psp.tile([128, 512], F32, tag="psP", bufs=2,
                              name=f"pj{dst[m].name}_{c0}")
                for k in range(KT):
                    nc.tensor.matmul(
                        ps[:, 0:c1 - c0],
                        ws(w_sb, k, m * 128, (m + 1) * 128),
                        xs(k, c0, c1),
                        start=(k == 0), stop=(k == KT - 1))
                nc.vector.tensor_scalar(dst[m][:, c0:c1],
                                        ps[:, 0:c1 - c0], b_sb[:, m:m + 1],
                                        None, ALU.add)

            def proj_v(sc):
                """One 128-key chunk of V: scale rows by e^mask, append the
                e^mask denominator column per head."""
                ps = psp.tile([128, 512], F32, tag="psP", bufs=2,
                              name=f"pv{sc}")
                for k in range(KT):
                    nc.tensor.matmul(
                        ps[:, 0:DL],
                        xs(k, sc * 128, (sc + 1) * 128),
                        ws(wv_sb, k, 0, DL),
                        start=(k == 0), stop=(k == KT - 1))
                v3 = v_t[sc].rearrange("p (h e) -> p h e", e=HDA)
                nc.vector.tensor_scalar(
                    v3[:, :, 0:HD],
                    ps[:, 0:DL].rearrange("p (h e) -> p h e", e=HD),
                    em_sb[:, sc:sc + 1], None, ALU.mult)
                nc.gpsimd.tensor_copy(
                    v3[:, :, HD:HDA],
                    em6_sb[:, sc * NH_LOC:(sc + 1) * NH_LOC].rearrange(
                        "p (h e) -> p h e", e=1))

            # serial front: K m0 kc0-3 and the first q half-window; the
            # second q half (Qn1) is emitted inside the warm start, after the
            # kc0-3 qq0 half-scores, so PE doesn't stall on the x 512:1024
            # DMA before giving ACT its first exp work.
            proj_qk(kT_t, wk_sb, bk_sb, 0, 0, 256)
            proj_qk(qT_t, wq_sb, bq_sb, 0, 0, 512)

            # deferred PE fillers, scheduled at explicit (unit, step) slots so
            # the early units stay close to the ACT exp pace. Deadlines:
            # K m0 n_i by u0 step 4i; Q m0 n2/n3 by u1 s0; V sc by u2 step sc
            # (ctx deferral = 2); K/Q m1 n0-n1 by u4 s0, K m1 n2/n3 by u4
            # s8/s12, Q m1 n2/n3 by u5 s0; m2 likewise shifted +4 units.
            QK = "qk"
            slots = {}

            def put(u, s, item):
                slots.setdefault((u, s), []).append(item)

            put(0, 1, (QK, kT_t, wk_sb, bk_sb, 0, 512, 1024))
            put(0, 4, (QK, kT_t, wk_sb, bk_sb, 0, 1024, 1536))
            put(0, 8, (QK, kT_t, wk_sb, bk_sb, 0, 1536, 2048))
            put(0, 11, (QK, qT_t, wq_sb, bq_sb, 0, 1024, 1536))
            put(0, 13, (QK, qT_t, wq_sb, bq_sb, 0, 1536, 2048))
            for sc in range(9):                   # V0-8 in u1
                put(1, 1 + sc * 7 // 4, ("v", sc))
            for sc in range(9, KC):               # V9-15 JIT in u2
                put(2, 2 * (sc - 9), ("v", sc))
            for m, base_u in [(1, 3), (2, 6)]:
                put(base_u, 2, (QK, kT_t, wk_sb, bk_sb, m, 0, 512))
                put(base_u, 6, (QK, kT_t, wk_sb, bk_sb, m, 512, 1024))
                put(base_u, 10, (QK, qT_t, wq_sb, bq_sb, m, 0, 512))
                put(base_u, 14, (QK, qT_t, wq_sb, bq_sb, m, 512, 1024))
                put(base_u + 1, 2, (QK, kT_t, wk_sb, bk_sb, m, 1024, 1536))
                put(base_u + 1, 6, (QK, kT_t, wk_sb, bk_sb, m, 1536, 2048))
                put(base_u + 1, 10, (QK, qT_t, wq_sb, bq_sb, m, 1024, 1536))
                put(base_u + 1, 14, (QK, qT_t, wq_sb, bq_sb, m, 1536, 2048))

            def drain_slot(u, s):
                for item in slots.pop((u, s), ()):
                    if item[0] == "v":
                        proj_v(item[1])
                    else:
                        _, dst, w_sb, b_sb, m, c0, c1 = item
                        proj_qk(dst, w_sb, b_sb, m, c0, c1)

            # ---- attention: ctx deferred 2 units ----
            units = [(h, qc) for h in range(NH_LOC) for qc in range(2)]
            DEFER = 2
            exp_tiles = {}
            ctx_ps = {}

            def _half(u, kc, qq, psS, eo):
                h, qc = units[u]
                m, off = divmod(h, 2)
                off *= HD
                q0 = qc * 1024
                hs_ = slice(qq * 512, (qq + 1) * 512)
                nc.tensor.matmul(
                    psS[:, hs_],
                    kT_t[m][off:off + HD, kc * 128:(kc + 1) * 128],
                    qT_t[m][off:off + HD,
                            q0 + qq * 512:q0 + (qq + 1) * 512],
                    start=True, stop=True)
                nc.scalar.activation(eo[:, hs_], psS[:, hs_], AFT.Exp,
                                     bias=abias[:, 0:1], scale=LN2)

            def _alloc_skc(u, kc):
                psS = psp.tile([128, 1024], F32, tag="psS", bufs=2,
                               name=f"psS{u}_{kc}")
                eo = wp.tile([128, 1024], BF16, tag="eS", bufs=2 * KC + 8,
                             name=f"eS{u}_{kc}")
                exp_tiles[(u, kc)] = eo
                return psS, eo

            def emit_scores(u, kc):
                h, qc = units[u]
                m, off = divmod(h, 2)
                off *= HD
                q0 = qc * 1024
                psS, eo = _alloc_skc(u, kc)
                for qq in range(2):
                    nc.tensor.matmul(
                        psS[:, qq * 512:(qq + 1) * 512],
                        kT_t[m][off:off + HD, kc * 128:(kc + 1) * 128],
                        qT_t[m][off:off + HD,
                                q0 + qq * 512:q0 + (qq + 1) * 512],
                        start=True, stop=True)
                nc.scalar.activation(eo[:], psS[:], AFT.Exp,
                                     bias=abias[:, 0:1], scale=LN2)

            def emit_ctx(u, kc, tag=None):
                h, qc = units[u]
                if kc == 0:
                    for bank in range(2):
                        ctx_ps[(u, bank)] = psp.tile(
                            [128, 512], F32,
                            tag=tag or f"ctx{bank}",
                            bufs=2 if tag else 1,
                            name=f"ctx{u}_{bank}")
                eo = exp_tiles.pop((u, kc))
                for j in range(8):
                    bank, jj = divmod(j, 4)
                    nc.tensor.matmul(
                        ctx_ps[(u, bank)][:, jj * 128:jj * 128 + HDA],
                        eo[:, j * 128:(j + 1) * 128],
                        v_t[kc][:, h * HDA:(h + 1) * HDA],
                        start=(kc == 0 and jj == 0), stop=(kc == KC - 1))

            def emit_drain(u):
                h, qc = units[u]
                q0 = qc * 1024
                osb = wp.tile([128, 8 * HDA], F32, tag="osb", bufs=2,
                              name=f"osb{u}")
                o3 = osb.rearrange("p (j e) -> p j e", e=HDA)
                dst = outT[q0:q0 + 1024, h * HDA:(h + 1) * HDA].rearrange(
                    "(j p) e -> p j e", p=128)
                for bank in range(2):
                    src = ctx_ps.pop((u, bank)).rearrange(
                        "p (j e) -> p j e", e=128)
                    bs = slice(bank * 4, (bank + 1) * 4)
                    nc.vector.tensor_copy(o3[:, bs, :], src[:, :, 0:HDA])
                    nc.sync.dma_start(dst[:, bs, :], o3[:, bs, :])

            # Units 0..NU-1 run scores; ctx of unit u normally runs during
            # u+2 (ctx0/ctx1 psum banks). Tail compression: units NU-2 and
            # NU-1 each run TWO ctx streams -- the second in the projection
            # psum banks (free after unit NU-5) -- so the final unit's ctx
            # trails its own exps by ~3 kc instead of a full unit.
            NU = len(units)
            TRAIL = 3
            # unit-0 warm start: kc0/kc1 scores+exp in q-halves so ACT gets
            # its first work before the second q window's x columns land.
            # unit-0 warm start: kc0-3 scores+exp in q halves so ACT starts
            # as soon as the first q half's x columns land. kc2/3 qq0 borrow
            # the ctx psum banks (idle until unit 2).
            warm_eo = []
            for kc in range(4):
                eo = wp.tile([128, 1024], BF16, tag="eS", bufs=2 * KC + 8,
                             name=f"eS0_{kc}")
                exp_tiles[(0, kc)] = eo
                warm_eo.append(eo)

            def warm_half(kc, qq, ps_ap):
                nc.tensor.matmul(
                    ps_ap,
                    kT_t[0][0:HD, kc * 128:(kc + 1) * 128],
                    qT_t[0][0:HD, qq * 512:(qq + 1) * 512],
                    start=True, stop=True)
                nc.scalar.activation(warm_eo[kc][:, qq * 512:(qq + 1) * 512],
                                     ps_ap, AFT.Exp, bias=abias[:, 0:1],
                                     scale=LN2)

            wps01 = [psp.tile([128, 1024], F32, tag="psS", bufs=2,
                              name=f"wps{kc}") for kc in range(2)]
            warm_half(0, 0, wps01[0][:, 0:512])
            warm_half(1, 0, wps01[1][:, 0:512])
            proj_qk(kT_t, wk_sb, bk_sb, 0, 256, 512)       # K for kc2/3
            proj_qk(qT_t, wq_sb, bq_sb, 0, 512, 1024)      # Qn1
            wc2 = psp.tile([128, 512], F32, tag="ctx0", bufs=1, name="wc2a")
            warm_half(2, 0, wc2[:])
            wc3 = psp.tile([128, 512], F32, tag="ctx1", bufs=1, name="wc3a")
            warm_half(3, 0, wc3[:])
            warm_half(0, 1, wps01[0][:, 512:1024])
            warm_half(1, 1, wps01[1][:, 512:1024])
            wc2b = psp.tile([128, 512], F32, tag="ctx0", bufs=1, name="wc2b")
            warm_half(2, 1, wc2b[:])
            wc3b = psp.tile([128, 512], F32, tag="ctx1", bufs=1, name="wc3b")
            warm_half(3, 1, wc3b[:])
            for u in range(NU):
                for kc in range(KC):
                    if u == 0 and kc < 4:
                        pass        # emitted in the warm start above
                    else:
                        emit_scores(u, kc)
                    if DEFER <= u < NU - 2:
                        emit_ctx(u - DEFER, kc)
                    elif u == NU - 2:
                        emit_ctx(u - 2, kc)             # ctx(NU-4) in ctx0/1
                        emit_ctx(u - 1, kc, tag="psP")  # ctx(NU-3) in psP
                    elif u == NU - 1:
                        emit_ctx(u - 1, kc)             # ctx(NU-2) in ctx0/1
                        if kc >= TRAIL:
                            emit_ctx(u, kc - TRAIL, tag="psP")
                    drain_slot(u, kc)
                if DEFER <= u < NU - 2:
                    emit_drain(u - DEFER)
                elif u == NU - 2:
                    emit_drain(u - 2)
                    emit_drain(u - 1)
                elif u == NU - 1:
                    emit_drain(u - 1)
            for kc in range(KC - TRAIL, KC):
                emit_ctx(NU - 1, kc, tag="psP")
            emit_drain(NU - 1)
            assert not slots, f"unscheduled fillers: {list(slots)}"

    nc.compile()
    return nc


def _get_nc():
    global _CACHED
    if _CACHED is None:
        _CACHED = _build()
    return _CACHED


def kernel(hidden_states, attention_mask, Wq, bq, Wk, bk, Wv, bv):
    hidden_states = np.asarray(hidden_states, np.float32)
    attention_mask = np.asarray(attention_mask, np.float32)
    Wq, Wk, Wv = (np.asarray(w, np.float32) for w in (Wq, Wk, Wv))
    bq, bk, bv = (np.asarray(b, np.float32) for b in (bq, bk, bv))

    nc = _get_nc()
    sc = np.float32(LOG2E / np.sqrt(HD))
    in_maps = []
    for core in range(N_CORES):
        b, g = divmod(core, 2)
        cs = slice(g * DL, (g + 1) * DL)
        # xp[p, k*S+s] = hidden[b][s, k*128+p]
        xT = hidden_states[b].T.reshape(KT, 128, S)           # [k, p, s]
        xp = np.ascontiguousarray(xT.transpose(1, 0, 2).reshape(128, KT * S))

        def wpack(Wmat, scale=None):
            wT = Wmat[cs, :].T                                 # [H, DL]
            if scale is not None:
                wT = wT * scale
            w3 = wT.reshape(KT, 128, DL).transpose(1, 0, 2)    # [p, k, c]
            return np.ascontiguousarray(w3.reshape(128, KT * DL)).astype(
                ml_dtypes.bfloat16)

        em = np.exp(attention_mask[b, 0, 0, :]).astype(np.float32)
        emc = np.ascontiguousarray(em.reshape(KC, 128).T)      # [128, KC]
        em6 = np.repeat(emc[:, :, None], NH_LOC, axis=2).reshape(
            128, KC * NH_LOC)
        auxm = np.concatenate([
            (bq[cs] * sc).reshape(M3, 128).T,
            bk[cs].reshape(M3, 128).T,
            emc, em6], axis=1).astype(np.float32)
        in_maps.append({
            "xp": xp.astype(ml_dtypes.bfloat16),
            "wqp": wpack(Wq, sc),
            "wkp": wpack(Wk),
            "wvp": wpack(Wv),
            "aux": np.ascontiguousarray(auxm),
        })

    res = run_bass_kernel_spmd(nc, in_maps, core_ids=list(range(N_CORES)))

    out = np.empty((B, S, H), np.float32)
    for core in range(N_CORES):
        b, g = divmod(core, 2)
        oT = res.results[core]["outT"]              # [2048, 390]
        o3 = oT.reshape(S, NH_LOC, HDA)
        ctx = o3[:, :, 0:HD] / o3[:, :, HD:HDA]     # [S, 6, 64]
        cols = slice(g * DL, (g + 1) * DL)
        out[b, :, cols] = ctx.reshape(S, DL) + bv[cols][None, :]
    return out
